# revision 1
# baseline (speedup 1.0000x reference)
"""3x3 same-conv (NHWC, 32x56x56x128 -> 32x56x56x256) + bias + ReLU on 8 TRN2 cores.

Strategy: data-parallel over batch (4 images/core). Per core, the conv is
9 shifted matmuls accumulated in PSUM with Cin=128 as the contraction dim:
  out[q, cout] = relu( sum_tap XpT[:, q+off_tap]^T @ W[tap] + b )
where XpT is the zero-padded image held transposed in SBUF ([cin, 58*58]
flat padded pixels, fp16), built once per image via PE transposes fed by a
casting SWDGE load. Each matmul group covers 116 contiguous padded
positions = two output rows plus 4 pad-junk anchors that are computed but
never stored; the junk keeps every matmul operand a contiguous SBUF window
(one free dim) while the two per-image output stores skip those partitions
with a strided DMA. fp16 operands stream the PE at 1 col/cycle with the
self-loading LDWEIGHTS fully hidden behind the previous matmul.
"""

import os
from contextlib import ExitStack

import numpy as np

import concourse.bass as bass
import concourse.bacc as bacc
import concourse.mybir as mybir
import concourse.tile as tile
from concourse.bass_utils import run_bass_kernel_spmd
from concourse.masks import make_identity

N_CORES = 8
B, H, W, CIN, COUT = 32, 56, 56, 128, 256
BPC = B // N_CORES            # images per core
S = W + 2                     # padded width (58)
PIMG = S * S                  # padded pixels per image (3364)
ANCH0 = S + 1                 # first valid anchor (59)
GROUPS = H // 2               # 28 row-pair anchor groups per image
GM = 2 * S                    # anchors per group (116: 2 padded rows, 4 junk)
SLAB_W = PIMG + 128           # per-image slab width incl. zero slop
RPC = 2                       # image rows per transpose chunk
CHUNK_PIX = RPC * W           # 112
NCHUNK = H // RPC             # 28

TAP_OFFS = [(dh - 1) * S + (dw - 1) for dh in range(3) for dw in range(3)]
F32 = mybir.dt.float32
F32R = mybir.dt.float32r
F16 = mybir.dt.float16

LAST_RESULTS = None


def _build(with_bias: bool):
    nc = bacc.Bacc("TRN2", target_bir_lowering=False, debug=False)
    x_h = nc.declare_dram_parameter("prev_a", [BPC, H, W, CIN], F32, isOutput=False)
    w_h = nc.declare_dram_parameter("filter_w", [3, 3, CIN, COUT], F32, isOutput=False)
    b_h = nc.declare_dram_parameter("filter_b", [1, 1, 1, COUT], F32, isOutput=False)
    y_h = nc.declare_dram_parameter("out", [BPC, H, W, COUT], F32, isOutput=True)
    x_ap, w_ap, b_ap, y_ap = x_h.ap(), w_h.ap(), b_h.ap(), y_h.ap()

    with tile.TileContext(nc) as tc, ExitStack() as ctx:
        const_pool = ctx.enter_context(tc.tile_pool(name="const", bufs=1))
        xslab_pool = ctx.enter_context(tc.tile_pool(name="xslab", bufs=1))
        stage_pool = ctx.enter_context(tc.tile_pool(name="stage", bufs=2))
        out_pool = ctx.enter_context(tc.tile_pool(name="outsb", bufs=2))
        psum_mm = ctx.enter_context(
            tc.tile_pool(name="psmm", bufs=4, space=bass.MemorySpace.PSUM)
        )
        psum_tp = ctx.enter_context(
            tc.tile_pool(name="pstp", bufs=4, space=bass.MemorySpace.PSUM)
        )

        # Weights: [3,3,128,256] -> SBUF [cin=128, tap*256], rounded to fp32r
        wstage = const_pool.tile([CIN, 9 * COUT], F32, tag="wstage")
        nc.sync.dma_start(
            out=wstage[:].rearrange("k (t n) -> k t n", t=9),
            in_=w_ap.rearrange("a b k n -> (a b) k n").transpose([1, 0, 2]),
        )
        wslab = const_pool.tile([CIN, 9 * COUT], F16, tag="wslab")
        nc.vector.tensor_copy(wslab[:], wstage[:])

        identity = const_pool.tile([CHUNK_PIX, CHUNK_PIX], F16, tag="ident")
        make_identity(nc, identity[:])

        if with_bias:
            bias_st = const_pool.tile([1, COUT], F32, tag="bias_st")
            nc.sync.dma_start(
                out=bias_st[:], in_=b_ap.rearrange("a b c n -> (a b c) n")
            )
            bias_sb = const_pool.tile([1, COUT], F16, tag="bias")
            nc.vector.tensor_copy(bias_sb[:], bias_st[:])
            ones_sb = const_pool.tile([1, 128], F16, tag="ones")
            nc.gpsimd.memset(ones_sb[:], 1.0)

        # Per-image transposed padded slabs [cin, 58*58 (+slop)]
        xslabs = []
        for i in range(BPC):
            sl = xslab_pool.tile([CIN, SLAB_W], F16, tag=f"xs{i}")
            xslabs.append(sl)
            nc.vector.memset(sl[:, 0:S], 0.0)  # top pad row
            nc.vector.memset(sl[:, (H + 1) * S : PIMG], 0.0)  # bottom pad row
            mid = sl[:, S : (H + 1) * S].rearrange("p (r c) -> p r c", c=S)
            nc.vector.memset(mid[:, :, 0:1], 0.0)  # left pad col
            nc.vector.memset(mid[:, :, S - 1 : S], 0.0)  # right pad col
            nc.vector.memset(sl[:, PIMG:SLAB_W], 0.0)  # slop

        # Slab build steps (load image, PE-transpose 2-row chunks, copy into
        # slab), emitted lazily so they interleave with prior image's matmuls
        def emit_load(i):
            stg = stage_pool.tile([CHUNK_PIX, NCHUNK * CIN], F16, tag="stage")
            src = (
                x_ap[i]
                .rearrange("h w c -> (h w) c")
                .rearrange("(n p) c -> n p c", p=CHUNK_PIX)
                .transpose([1, 0, 2])
            )
            dstv = stg[:].rearrange("p (n c) -> p n c", n=NCHUNK)
            for c0 in range(0, NCHUNK, 7):
                c1 = min(c0 + 7, NCHUNK)
                nc.gpsimd.dma_start(out=dstv[:, c0:c1, :], in_=src[:, c0:c1, :])
            return stg

        def emit_transpose(i, stg, cidx):
            pst = psum_tp.tile([CIN, CHUNK_PIX], F16, tag="pst")
            nc.tensor.transpose(
                pst[:], stg[:, cidx * CIN : (cidx + 1) * CIN], identity[:]
            )
            dst = (
                xslabs[i][:, (RPC * cidx + 1) * S : (RPC * cidx + 1 + RPC) * S]
                .rearrange("p (r c) -> p r c", c=S)[:, :, 1 : 1 + W]
            )
            nc.scalar.activation(
                dst,
                pst[:].rearrange("p (r c) -> p r c", c=W),
                mybir.ActivationFunctionType.Copy,
            )

        def emit_group(i, g, oslab):
            # anchors = 116 contiguous padded positions covering output rows
            # (2g, 2g+1); partitions 56,57,114,115 are pad junk (never stored)
            q0 = (2 * g + 1) * S + 1
            ps = psum_mm.tile([GM, COUT], F32, tag="psmm")
            for t in range(9):
                w0 = q0 + TAP_OFFS[t]
                nc.tensor.matmul(
                    ps[:],
                    xslabs[i][:, w0 : w0 + GM],
                    wslab[:, t * COUT : (t + 1) * COUT],
                    start=(t == 0),
                    stop=(t == 8 and not with_bias),
                )
            if with_bias:
                nc.tensor.matmul(
                    ps[:], ones_sb[:1, :GM], bias_sb[:1, :], start=False, stop=True
                )
            nc.vector.tensor_scalar_max(
                oslab[:, g * COUT : (g + 1) * COUT], ps[:], 0.0
            )

        # Image 0's slab is built up front; image i+1's transposes are
        # interleaved between image i's matmul groups so the PE never waits
        # on a bulk transpose phase.
        stg0 = emit_load(0)
        nxt = emit_load(1)
        for c in range(NCHUNK):
            emit_transpose(0, stg0, c)
        for i in range(BPC):
            oslab = out_pool.tile([GM, GROUPS * COUT], F32, tag="osb")
            done = 0
            for g in range(GROUPS):
                emit_group(i, g, oslab)
                if i + 1 < BPC:
                    # spread the 28 transposes of image i+1 over the groups
                    want = (g + 1) * NCHUNK // GROUPS
                    while done < want:
                        emit_transpose(i + 1, nxt, done)
                        done += 1
            # SWDGE stores: partitions 0-55 = even rows, 58-113 = odd rows;
            # issued in group-chunks so they overlap the remaining compute
            dst_all = y_ap[i].rearrange("(g r) w c -> r w g c", r=2)
            for r in range(2):
                srcv = oslab[r * S : r * S + W, :].rearrange(
                    "p (g c) -> p g c", g=GROUPS
                )
                for q0 in range(0, GROUPS, 4):
                    q1 = min(q0 + 4, GROUPS)
                    nc.gpsimd.dma_start(
                        out=dst_all[r][:, q0:q1, :], in_=srcv[:, q0:q1, :]
                    )
            if i + 1 < BPC:
                while done < NCHUNK:
                    emit_transpose(i + 1, nxt, done)
                    done += 1
                if i + 2 < BPC:
                    nxt = emit_load(i + 2)

    nc.compile()
    return nc


_CACHE = {}


def _get_nc(with_bias: bool):
    if with_bias not in _CACHE:
        _CACHE[with_bias] = _build(with_bias)
    return _CACHE[with_bias]


def kernel(prev_a, filter_w, filter_b):
    global LAST_RESULTS
    prev_a = np.ascontiguousarray(prev_a, dtype=np.float32)
    filter_w = np.ascontiguousarray(filter_w, dtype=np.float32)
    filter_b = np.ascontiguousarray(filter_b, dtype=np.float32).reshape(1, 1, 1, COUT)
    with_bias = bool(np.any(filter_b))
    nc = _get_nc(with_bias)
    in_maps = [
        {
            "prev_a": prev_a[c * BPC : (c + 1) * BPC],
            "filter_w": filter_w,
            "filter_b": filter_b,
        }
        for c in range(N_CORES)
    ]
    trace = os.environ.get("KERNEL_TRACE") == "1"
    res = run_bass_kernel_spmd(nc, in_maps, list(range(N_CORES)), trace=trace)
    LAST_RESULTS = res
    return np.concatenate([res.results[c]["out"] for c in range(N_CORES)], axis=0)



# revision 4
# speedup vs baseline: 1.2718x; 1.2718x over previous
"""3x3 same-conv (NHWC, 32x56x56x128 -> 32x56x56x256) + bias + ReLU on 8 TRN2 cores.

Strategy: data-parallel over batch (4 images/core). Per core, the conv is
9 shifted matmuls accumulated in PSUM with Cin=128 as the contraction dim:
  out[q, cout] = relu( sum_tap XpT[:, q+off_tap]^T @ W[tap] + b )

The padded image is held transposed in SBUF as [cin=128, 57*58-ish] fp16
where rows use a width-57 layout: each row is 56 data pixels plus ONE
shared zero column that serves as both the right pad of row r and the
left pad of row r+1. That makes the anchor stream 57 positions per row
(1.9% junk) instead of 58 (3.4%), and lets anchor groups be exactly 128
wide: M=128 stationary operands enable the PE's Fast Weight Load path,
hiding LDWEIGHTS (the baseline M=116 groups disabled FWL and paced the
whole kernel at LDWEIGHTS speed).

Group g covers anchors [128g, 128g+128) of the per-image anchor space
(3200 anchors = 25 groups; anchor m = 57*r + c, valid iff c < 56). The
output is stored anchor-padded to DRAM ([3200, 256] per image, one big
contiguous DMA per half-image) and de-padded on the host with a numpy
slice - that keeps every device-side store large and regular.
"""

import os
from contextlib import ExitStack

import numpy as np

import concourse.bass as bass
import concourse.bacc as bacc
import concourse.mybir as mybir
import concourse.tile as tile
from concourse.bass_utils import run_bass_kernel_spmd
from concourse.masks import make_identity

N_CORES = 8
B, H, W, CIN, COUT = 32, 56, 56, 128, 256
BPC = B // N_CORES            # images per core
RS = W + 1                    # row stride in the slab (57: 56 data + 1 shared pad)
A0 = RS + 1                   # slab position of anchor 0 = pixel (0,0)
NG = 25                       # anchor groups per image (25*128 = 3200 >= 56*57)
NANCH = NG * 128              # padded anchors per image
SLABW = 3328                  # slab width: >= A0 + NANCH + RS + 1, padded
RPC = 2                       # image rows per transpose chunk
CHUNK_PIX = RPC * W           # 112
NCHUNK = H // RPC             # 28
GH0 = 13                      # groups in first store half (12 in second)

TAP_OFFS = [(dh - 1) * RS + (dw - 1) for dh in range(3) for dw in range(3)]
F32 = mybir.dt.float32
F16 = mybir.dt.float16

LAST_RESULTS = None


def _build(with_bias: bool):
    nc = bacc.Bacc("TRN2", target_bir_lowering=False, debug=False)
    x_h = nc.declare_dram_parameter("prev_a", [BPC, H, W, CIN], F32, isOutput=False)
    w_h = nc.declare_dram_parameter("filter_w", [3, 3, CIN, COUT], F32, isOutput=False)
    b_h = nc.declare_dram_parameter("filter_b", [1, 1, 1, COUT], F32, isOutput=False)
    y_h = nc.declare_dram_parameter("out", [BPC, NANCH, COUT], F32, isOutput=True)
    x_ap, w_ap, b_ap, y_ap = x_h.ap(), w_h.ap(), b_h.ap(), y_h.ap()

    with tile.TileContext(nc) as tc, ExitStack() as ctx:
        const_pool = ctx.enter_context(tc.tile_pool(name="const", bufs=1))
        xslab_pool = ctx.enter_context(tc.tile_pool(name="xslab", bufs=1))
        stage_pool = ctx.enter_context(tc.tile_pool(name="stage", bufs=2))
        out_pool = ctx.enter_context(tc.tile_pool(name="outsb", bufs=3))
        psum_mm = ctx.enter_context(
            tc.tile_pool(name="psmm", bufs=6, space=bass.MemorySpace.PSUM)
        )
        psum_tp = ctx.enter_context(
            tc.tile_pool(name="pstp", bufs=2, space=bass.MemorySpace.PSUM)
        )

        # Weights: [3,3,128,256] -> SBUF [cin=128, tap*256] fp16
        wstage = const_pool.tile([CIN, 9 * COUT], F32, tag="wstage")
        nc.sync.dma_start(
            out=wstage[:].rearrange("k (t n) -> k t n", t=9),
            in_=w_ap.rearrange("a b k n -> (a b) k n").transpose([1, 0, 2]),
        )
        wslab = const_pool.tile([CIN, 9 * COUT], F16, tag="wslab")
        nc.vector.tensor_copy(wslab[:], wstage[:])

        identity = const_pool.tile([CHUNK_PIX, CHUNK_PIX], F16, tag="ident")
        make_identity(nc, identity[:])

        if with_bias:
            bias_st = const_pool.tile([1, COUT], F32, tag="bias_st")
            nc.sync.dma_start(
                out=bias_st[:], in_=b_ap.rearrange("a b c n -> (a b c) n")
            )
            bias_sb = const_pool.tile([1, COUT], F16, tag="bias")
            nc.vector.tensor_copy(bias_sb[:], bias_st[:])
            ones_sb = const_pool.tile([1, 128], F16, tag="ones")
            nc.gpsimd.memset(ones_sb[:], 1.0)

        # Per-image transposed padded slabs [cin, SLABW] fp16, width-57 rows
        xslabs = []
        for i in range(BPC):
            sl = xslab_pool.tile([CIN, SLABW], F16, tag=f"xs{i}")
            xslabs.append(sl)
            nc.vector.memset(sl[:, 0:A0], 0.0)  # top pad row + row0 left pad
            # shared pad column between consecutive rows (r=0..54)
            mid = sl[:, A0 + W : A0 + W + 55 * RS].rearrange(
                "p (r c) -> p r c", c=RS
            )
            nc.vector.memset(mid[:, :, 0:1], 0.0)
            # bottom pad row + group-tail slop
            nc.vector.memset(sl[:, A0 + 55 * RS + W : SLABW], 0.0)

        # Slab build steps (load image, PE-transpose 2-row chunks, copy into
        # slab), emitted lazily so they interleave with matmul groups
        def emit_load(i):
            stg = stage_pool.tile([CHUNK_PIX, NCHUNK * CIN], F16, tag="stage")
            src = (
                x_ap[i]
                .rearrange("h w c -> (h w) c")
                .rearrange("(n p) c -> n p c", p=CHUNK_PIX)
                .transpose([1, 0, 2])
            )
            dstv = stg[:].rearrange("p (n c) -> p n c", n=NCHUNK)
            for c0 in range(0, NCHUNK, 7):
                c1 = min(c0 + 7, NCHUNK)
                nc.gpsimd.dma_start(out=dstv[:, c0:c1, :], in_=src[:, c0:c1, :])
            return stg

        def emit_transpose(i, stg, cidx):
            pst = psum_tp.tile([CIN, CHUNK_PIX], F16, tag="pst")
            nc.tensor.transpose(
                pst[:], stg[:, cidx * CIN : (cidx + 1) * CIN], identity[:]
            )
            dst = (
                xslabs[i][:, A0 + RPC * cidx * RS : A0 + (RPC * cidx + RPC) * RS]
                .rearrange("p (r c) -> p r c", c=RS)[:, :, 0:W]
            )
            nc.scalar.activation(
                dst,
                pst[:].rearrange("p (r c) -> p r c", c=W),
                mybir.ActivationFunctionType.Copy,
            )

        def emit_group(i, g, oslab, o0):
            # 128 anchors [128g, 128g+128); anchor m = 57r + c, junk iff c=56
            q0 = A0 + 128 * g
            ps = psum_mm.tile([128, COUT], F32, tag="psmm")
            for t in range(9):
                w0 = q0 + TAP_OFFS[t]
                nc.tensor.matmul(
                    ps[:],
                    xslabs[i][:, w0 : w0 + 128],
                    wslab[:, t * COUT : (t + 1) * COUT],
                    start=(t == 0),
                    stop=(t == 8 and not with_bias),
                )
            if with_bias:
                nc.tensor.matmul(
                    ps[:], ones_sb[:1, :128], bias_sb[:1, :], start=False, stop=True
                )
            nc.vector.tensor_scalar_max(
                oslab[:, (g - o0) * COUT : (g - o0 + 1) * COUT], ps[:], 0.0
            )

        def emit_store(i, oslab, g0, g1):
            dst = y_ap[i].rearrange("(g p) c -> p g c", p=128)[:, g0:g1, :]
            nc.sync.dma_start(
                out=dst, in_=oslab[:].rearrange("p (g c) -> p g c", g=g1 - g0)
            )

        # Pipeline: image 0's first two transposes up front, then its groups
        # with image-0 (then image-1) transposes interleaved between groups so
        # the PE starts matmuls early and never waits on a bulk phase.
        stg_cur = emit_load(0)
        stg_nxt = emit_load(1) if BPC > 1 else None
        emit_transpose(0, stg_cur, 0)
        emit_transpose(0, stg_cur, 1)
        done_cur = 2  # transposes emitted for current image
        done_nxt = 0  # transposes emitted for next image
        for i in range(BPC):
            halves = [(0, GH0), (GH0, NG)]
            for g0, g1 in halves:
                oslab = out_pool.tile([128, (g1 - g0) * COUT], F32, tag="osb")
                for g in range(g0, g1):
                    # current image's own remaining transposes must stay ahead
                    # of the groups that read them: group g's max slab position
                    # is A0+128g+127+58, i.e. data row (128g+185)//57
                    need = min(NCHUNK, (128 * g + 185) // RS // RPC + 1)
                    while done_cur < need:
                        emit_transpose(i, stg_cur, done_cur)
                        done_cur += 1
                    emit_group(i, g, oslab, g0)
                    if done_cur >= NCHUNK and i + 1 < BPC:
                        # spread next image's transposes over remaining groups
                        want = (g + 1) * NCHUNK // NG
                        while done_nxt < min(want, NCHUNK):
                            emit_transpose(i + 1, stg_nxt, done_nxt)
                            done_nxt += 1
                emit_store(i, oslab, g0, g1)
            if i + 1 < BPC:
                while done_nxt < NCHUNK:
                    emit_transpose(i + 1, stg_nxt, done_nxt)
                    done_nxt += 1
                stg_cur, done_cur = stg_nxt, NCHUNK
                done_nxt = 0
                if i + 2 < BPC:
                    stg_nxt = emit_load(i + 2)

    nc.compile()
    return nc


_CACHE = {}


def _get_nc(with_bias: bool):
    if with_bias not in _CACHE:
        _CACHE[with_bias] = _build(with_bias)
    return _CACHE[with_bias]


def kernel(prev_a, filter_w, filter_b):
    global LAST_RESULTS
    prev_a = np.ascontiguousarray(prev_a, dtype=np.float32)
    filter_w = np.ascontiguousarray(filter_w, dtype=np.float32)
    filter_b = np.ascontiguousarray(filter_b, dtype=np.float32).reshape(1, 1, 1, COUT)
    with_bias = bool(np.any(filter_b))
    nc = _get_nc(with_bias)
    in_maps = [
        {
            "prev_a": prev_a[c * BPC : (c + 1) * BPC],
            "filter_w": filter_w,
            "filter_b": filter_b,
        }
        for c in range(N_CORES)
    ]
    trace = os.environ.get("KERNEL_TRACE") == "1"
    res = run_bass_kernel_spmd(nc, in_maps, list(range(N_CORES)), trace=trace)
    LAST_RESULTS = res
    # de-pad: anchor m = 57*r + c, valid iff c < 56
    outs = []
    for c in range(N_CORES):
        ypad = res.results[c]["out"]
        outs.append(
            ypad[:, : H * RS, :].reshape(BPC, H, RS, COUT)[:, :, :W, :]
        )
    return np.ascontiguousarray(np.concatenate(outs, axis=0))
